# revision 2
# baseline (speedup 1.0000x reference)
"""Trainium2 Bass kernel for the pairwise concordance-index loss.

reference:
    loss = sum_{i<j, f_i=f_j=1} relu((p_i-p_j)(t_i-t_j)) / 100 / n_pairs

Math used here:
  M[i,j] = f_i f_j (p_i-p_j)(t_i-t_j) = A^T B  with rank 4:
      A = [f*u, f, f*p, f*t],  B = [f, f*u, -f*t, -f*p],  u = p*t
  (flags fold in because relu(f_i f_j x) = f_i f_j relu(x) for 0/1 flags)
  sum relu(M) = 0.5*(sum M + sum |M|); sum M has an O(B) closed form done
  on the host in fp64; sum |M| is the O(B^2) part done on device.

Device work decomposition (8 cores, identical program, data-sharded):
  64 row-blocks of 128. Core k owns row-blocks 8k..8k+7. For each owned
  row-block a, it processes column-blocks at cyclic offsets e=0..32
  (cols 128a+128e .. +128, mod 8192). Offsets e=1..31 carry weight 1,
  e=0 (the symmetric diagonal square) and e=32 (covered twice globally)
  carry weight 0.5. This covers every unordered pair exactly once.

Per row-block the device does K=4 matmuls (TensorE) generating PSUM tiles
of M, then abs-row-sums them with DVE tensor_reduce(apply_absolute_value)
and ScalarE activation(Abs, accum_out) split across both engines.
"""

import numpy as np

B = 8192
P = 128
NCORE = 8
ABLK = 8          # row-blocks per core
BCOLS = 5120      # per-core column-slab width: 128*(7 + 33)
NJOB = 5          # reduce jobs per row-block (T1..T4, D)
NACC = ABLK * NJOB  # 40 accumulator columns

_cache = {}


def _build():
    """Build + compile the Bass module (once per process)."""
    import concourse.bacc as bacc
    import concourse.tile as tile
    import concourse.mybir as mybir

    f32 = mybir.dt.float32
    nc = bacc.Bacc("TRN2", target_bir_lowering=False, debug=False, num_devices=NCORE)

    a_dram = nc.dram_tensor("a_rows", [4, P * ABLK], f32, kind="ExternalInput")
    b_dram = nc.dram_tensor("b_cols", [4, BCOLS], f32, kind="ExternalInput")
    acc_dram = nc.dram_tensor("acc", [2, P, NACC], f32, kind="ExternalOutput")

    with tile.TileContext(nc) as tc:
        with (
            tc.tile_pool(name="inp", bufs=1) as inp_pool,
            tc.tile_pool(name="accp", bufs=1) as acc_pool,
            tc.tile_pool(name="psT", bufs=3, space="PSUM") as psT,
            tc.tile_pool(name="psD", bufs=2, space="PSUM") as psD,
        ):
            a_sb = inp_pool.tile([4, P * ABLK], f32)
            b_sb = inp_pool.tile([4, BCOLS], f32)
            # chunk the input DMAs so they spread across DMA queues
            for c in range(2):
                w = P * ABLK // 2
                nc.sync.dma_start(
                    a_sb[:, c * w:(c + 1) * w], a_dram.ap()[:, c * w:(c + 1) * w]
                )
            for c in range(8):
                w = BCOLS // 8
                nc.sync.dma_start(
                    b_sb[:, c * w:(c + 1) * w], b_dram.ap()[:, c * w:(c + 1) * w]
                )

            acc_dve = acc_pool.tile([P, NACC], f32)
            acc_act = acc_pool.tile([P, NACC], f32)
            nc.vector.memset(acc_dve[:, :], 0.0)
            nc.scalar.memzero(acc_act[:, :])

            for a in range(ABLK):
                base = P * a
                lhsT = a_sb[0:4, base:base + P]
                # (psum_width, [(col_off, n, psum_off), ...], acc_col, weight_tag)
                jobs = [
                    # T1..T4: e=1..31, weight 1
                    (1024, [(base + 128, 512, 0), (base + 640, 512, 512)], NJOB * a + 0),
                    (1024, [(base + 1152, 512, 0), (base + 1664, 512, 512)], NJOB * a + 1),
                    (1024, [(base + 2176, 512, 0), (base + 2688, 512, 512)], NJOB * a + 2),
                    (896, [(base + 3200, 512, 0), (base + 3712, 384, 512)], NJOB * a + 3),
                    # D: e=0 and e=32, weight 0.5 (applied on host)
                    (256, [(base, 128, 0), (base + 4096, 128, 128)], NJOB * a + 4),
                ]
                for j, (width, mms, acc_col) in enumerate(jobs):
                    pool = psD if width <= 512 else psT
                    mtile = pool.tile([P, width], f32, tag="d" if width <= 512 else "t")
                    for (off, n, poff) in mms:
                        nc.tensor.matmul(
                            mtile[:, poff:poff + n],
                            lhsT,
                            b_sb[0:4, off:off + n],
                            start=True,
                            stop=True,
                        )
                    # split reduces across DVE and ACT, alternating per row-block
                    use_dve = (j % 2 == 0) if (a % 2 == 0) else (j % 2 == 1)
                    if use_dve:
                        nc.vector.tensor_reduce(
                            acc_dve[:, acc_col:acc_col + 1],
                            mtile[:, :],
                            axis=mybir.AxisListType.X,
                            op=mybir.AluOpType.add,
                            apply_absolute_value=True,
                        )
                    else:
                        nc.scalar.activation(
                            mtile[:, :],
                            mtile[:, :],
                            mybir.ActivationFunctionType.Abs,
                            accum_out=acc_act[:, acc_col:acc_col + 1],
                        )

            nc.sync.dma_start(acc_dram.ap()[0], acc_dve[:, :])
            nc.sync.dma_start(acc_dram.ap()[1], acc_act[:, :])

    nc.compile()
    return nc


def _get_nc():
    if "nc" not in _cache:
        _cache["nc"] = _build()
    return _cache["nc"]


def kernel(pred, gt, gt_fracTime, gt_ifMOF):
    from concourse import bass_utils

    pred = np.asarray(pred)
    gt = np.asarray(gt)
    ift = int(np.asarray(gt_fracTime))
    imf = int(np.asarray(gt_ifMOF))

    p = pred.astype(np.float32)
    t = gt[:, ift].astype(np.float32)
    f = (gt[:, imf] == 1).astype(np.float32)
    u = (p * t).astype(np.float32)

    A = np.ascontiguousarray(np.stack([f * u, f, f * p, f * t]).astype(np.float32))
    Bm = np.ascontiguousarray(
        np.stack([f, f * u, -f * t, -f * p]).astype(np.float32)
    )

    in_maps = []
    for k in range(NCORE):
        a_rows = np.ascontiguousarray(A[:, 1024 * k:1024 * k + 1024])
        cols = (1024 * k + np.arange(BCOLS)) % B
        b_cols = np.ascontiguousarray(Bm[:, cols])
        in_maps.append({"a_rows": a_rows, "b_cols": b_cols})

    nc = _get_nc()
    res = bass_utils.run_bass_kernel_spmd(nc, in_maps, core_ids=list(range(NCORE)))

    # gather: acc[:, :, c] where c % NJOB == 4 are the 0.5-weight diag jobs
    T = 0.0
    for r in res.results:
        acc = r["acc"].astype(np.float64)  # [2, P, NACC]
        w = np.ones(NACC)
        w[NJOB - 1::NJOB] = 0.5
        T += (acc.sum(axis=(0, 1)) * w).sum()

    # host closed form in fp64
    f64 = f.astype(np.float64)
    S_f = f64.sum()
    S_fu = (f64 * u.astype(np.float64)).sum()
    S_fp = (f64 * p.astype(np.float64)).sum()
    S_ft = (f64 * t.astype(np.float64)).sum()
    S_half = S_fu * S_f - S_fp * S_ft
    n_pairs = (S_f * S_f - S_f) / 2.0

    loss = 0.5 * (S_half + T) / 100.0 / n_pairs
    return np.asarray(np.float32(loss))


# revision 4
# speedup vs baseline: 2.8719x; 2.8719x over previous
"""Trainium2 Bass kernel for the pairwise concordance-index loss.

reference:
    loss = sum_{i<j, f_i=f_j=1} relu((p_i-p_j)(t_i-t_j)) / 100 / n_pairs

Math used here:
  M[i,j] = f_i f_j (p_i-p_j)(t_i-t_j) = A^T B  with rank 4:
      A = [f*u, f, f*p, f*t],  B = [f, f*u, -f*t, -f*p],  u = p*t
  (flags fold in because relu(f_i f_j x) = f_i f_j relu(x) for 0/1 flags)
  sum relu(M) = 0.5*(sum M + sum |M|); sum M has an O(B) closed form done
  on the host in fp64; sum |M| is the O(B^2) part done on device.

Device work decomposition (8 cores, identical program, data-sharded):
  64 row-blocks of 128. Core k owns row-blocks 8k..8k+7. For each owned
  row-block a, it processes column-blocks at cyclic offsets e=0..32
  (cols 128a+128e .. +128, mod 8192). Offsets e=1..31 carry weight 1,
  e=0 (the symmetric diagonal square) and e=32 (covered twice globally)
  carry weight 0.5. This covers every unordered pair exactly once.

Per row-block the device does K=4 matmuls (TensorE) generating PSUM tiles
of M, then abs-row-sums them with DVE tensor_reduce(apply_absolute_value)
and ScalarE activation(Abs, accum_out) split across both engines.
"""

import numpy as np

B = 8192
P = 128
NCORE = 8
ABLK = 8          # row-blocks per core
BCOLS = 5120      # per-core column-slab width: 128*(7 + 33)
NJOB = 5          # reduce jobs per row-block (T1..T4, D)
NACC = ABLK * NJOB  # 40 accumulator columns

_cache = {}


def _build():
    """Build + compile the Bass module (once per process)."""
    import concourse.bacc as bacc
    import concourse.tile as tile
    import concourse.mybir as mybir

    f32 = mybir.dt.float32
    bf16 = mybir.dt.bfloat16
    nc = bacc.Bacc("TRN2", target_bir_lowering=False, debug=False, num_devices=NCORE)

    a_dram = nc.dram_tensor("a_rows", [4, P * ABLK], bf16, kind="ExternalInput")
    b_dram = nc.dram_tensor("b_cols", [4, BCOLS], bf16, kind="ExternalInput")
    acc_dram = nc.dram_tensor("acc", [2, P, NACC], f32, kind="ExternalOutput")

    with tile.TileContext(nc) as tc:
        with (
            tc.tile_pool(name="inp", bufs=1) as inp_pool,
            tc.tile_pool(name="accp", bufs=1) as acc_pool,
            tc.tile_pool(name="psT", bufs=3, space="PSUM") as psT,
            tc.tile_pool(name="psD", bufs=2, space="PSUM") as psD,
        ):
            a_sb = inp_pool.tile([4, P * ABLK], bf16)
            b_sb = inp_pool.tile([4, BCOLS], bf16)
            # chunk the input DMAs so they spread across DMA queues
            for c in range(2):
                w = P * ABLK // 2
                nc.sync.dma_start(
                    a_sb[:, c * w:(c + 1) * w], a_dram.ap()[:, c * w:(c + 1) * w]
                )
            for c in range(8):
                w = BCOLS // 8
                nc.sync.dma_start(
                    b_sb[:, c * w:(c + 1) * w], b_dram.ap()[:, c * w:(c + 1) * w]
                )

            acc_dve = acc_pool.tile([P, NACC], f32)
            acc_act = acc_pool.tile([P, NACC], f32)
            nc.vector.memset(acc_dve[:, :], 0.0)
            nc.scalar.memzero(acc_act[:, :])

            for a in range(ABLK):
                base = P * a
                lhsT = a_sb[0:4, base:base + P]
                # (psum_width, [(col_off, n, psum_off), ...], acc_col, weight_tag)
                jobs = [
                    # T1..T4: e=1..31, weight 1
                    (1024, [(base + 128, 512, 0), (base + 640, 512, 512)], NJOB * a + 0),
                    (1024, [(base + 1152, 512, 0), (base + 1664, 512, 512)], NJOB * a + 1),
                    (1024, [(base + 2176, 512, 0), (base + 2688, 512, 512)], NJOB * a + 2),
                    (896, [(base + 3200, 512, 0), (base + 3712, 384, 512)], NJOB * a + 3),
                    # D: e=0 and e=32, weight 0.5 (applied on host)
                    (256, [(base, 128, 0), (base + 4096, 128, 128)], NJOB * a + 4),
                ]
                for j, (width, mms, acc_col) in enumerate(jobs):
                    pool = psD if width <= 512 else psT
                    mtile = pool.tile([P, width], f32, tag="d" if width <= 512 else "t")
                    for (off, n, poff) in mms:
                        nc.tensor.matmul(
                            mtile[:, poff:poff + n],
                            lhsT,
                            b_sb[0:4, off:off + n],
                            start=True,
                            stop=True,
                        )
                    # split reduces across DVE and ACT, alternating per row-block
                    use_dve = (j % 2 == 0) if (a % 2 == 0) else (j % 2 == 1)
                    if use_dve:
                        nc.vector.tensor_reduce(
                            acc_dve[:, acc_col:acc_col + 1],
                            mtile[:, :],
                            axis=mybir.AxisListType.X,
                            op=mybir.AluOpType.add,
                            apply_absolute_value=True,
                        )
                    else:
                        nc.scalar.activation(
                            mtile[:, :],
                            mtile[:, :],
                            mybir.ActivationFunctionType.Abs,
                            accum_out=acc_act[:, acc_col:acc_col + 1],
                        )

            nc.sync.dma_start(acc_dram.ap()[0], acc_dve[:, :])
            nc.sync.dma_start(acc_dram.ap()[1], acc_act[:, :])

    nc.compile()
    return nc


def _get_nc():
    if "nc" not in _cache:
        _cache["nc"] = _build()
    return _cache["nc"]


def kernel(pred, gt, gt_fracTime, gt_ifMOF):
    from concourse import bass_utils

    pred = np.asarray(pred)
    gt = np.asarray(gt)
    ift = int(np.asarray(gt_fracTime))
    imf = int(np.asarray(gt_ifMOF))

    p = pred.astype(np.float32)
    t = gt[:, ift].astype(np.float32)
    f = (gt[:, imf] == 1).astype(np.float32)
    u = (p * t).astype(np.float32)

    import ml_dtypes

    A = np.ascontiguousarray(
        np.stack([f * u, f, f * p, f * t]).astype(ml_dtypes.bfloat16)
    )
    Bm = np.ascontiguousarray(
        np.stack([f, f * u, -f * t, -f * p]).astype(ml_dtypes.bfloat16)
    )

    in_maps = []
    for k in range(NCORE):
        a_rows = np.ascontiguousarray(A[:, 1024 * k:1024 * k + 1024])
        cols = (1024 * k + np.arange(BCOLS)) % B
        b_cols = np.ascontiguousarray(Bm[:, cols])
        in_maps.append({"a_rows": a_rows, "b_cols": b_cols})

    nc = _get_nc()
    res = bass_utils.run_bass_kernel_spmd(nc, in_maps, core_ids=list(range(NCORE)))

    # gather: acc[:, :, c] where c % NJOB == 4 are the 0.5-weight diag jobs
    T = 0.0
    for r in res.results:
        acc = r["acc"].astype(np.float64)  # [2, P, NACC]
        w = np.ones(NACC)
        w[NJOB - 1::NJOB] = 0.5
        T += (acc.sum(axis=(0, 1)) * w).sum()

    # host closed form in fp64, over the same bf16 values the device used:
    # sum_{i<j} M = (sum_{i,j} M - sum_diag M) / 2, with
    # sum_{i,j} M = sum_k (sum_i A_k)(sum_j B_k)
    A64 = A.astype(np.float64)
    B64 = Bm.astype(np.float64)
    S_all = (A64.sum(axis=1) * B64.sum(axis=1)).sum()
    D_diag = (A64 * B64).sum()
    S_half = (S_all - D_diag) / 2.0

    f64 = f.astype(np.float64)
    S_f = f64.sum()
    n_pairs = (S_f * S_f - S_f) / 2.0

    loss = 0.5 * (S_half + T) / 100.0 / n_pairs
    return np.asarray(np.float32(loss))
